# revision 38
# baseline (speedup 1.0000x reference)
"""DEMONet 3-layer GNN message-passing kernel for 8x Trainium2 NeuronCores.

Math per layer (verified against reference; all nodes have deg > 0):
    nm   = segment_sum(h[dst], src) / deg
    out  = elu(h @ (Wg + Ws) + nm @ Wl + b)          (b == 0 here)

Final design (5.60ms baseline -> 1.38ms):
  * Nodes row-partitioned across 8 cores (12800 padded/core); edges live
    with their src node's core; supertiles of SPT=5 tiles (128 nodes each).
  * Gather table for layer L holds rows (h_{L-1} @ Wl_L) in bf16 padded to
    128 cols (256B rows, pad never read), so the on-device segment-sum
    directly yields nm @ Wl.  Layer 0's table (x @ Wl1) comes from the host.
  * Neighbour rows fetched with gpsimd.dma_gather, <=1024 rows per call
    (hard HW limit), int16 indices relative to one of NB=4 bucket windows
    (25600 rows < 32768).  queue_num selects the Q7 core pair that runs the
    descgen ucode: 4 queues run concurrently (4x), with a pairing pattern
    that gives every queue one 1024- and one 896-idx gather per supertile.
  * Buckets are supertile-group slices of the table ([group][core][rows]
    layout, iteratively rebalanced group membership to keep the packing
    pools feasible), so each layer's AllGather splits into 4 slice
    collectives that fire as their group completes and overlap compute.
  * Segment-sum on the TensorEngine: S_ch [128e,128n] (0/1, built on the
    HOST, shipped as input) x X_ch [128e,64] accumulate PSUM per tile.
  * h kept transposed [64, NPC] bf16 in SBUF across layers; per tile:
    g-term matmul, (SU*invdeg)+g, supertile-batched ELU, transpose to
    update hT, and next-layer table-row matmul.
"""

import os
import numpy as np
import ml_dtypes

import concourse.bass as bass
import concourse.bacc as bacc
import concourse.mybir as mybir
import concourse.tile as tile
from concourse.bass_utils import run_bass_kernel_spmd
from concourse.masks import make_identity

F32 = mybir.dt.float32
BF16 = mybir.dt.bfloat16
I32 = mybir.dt.int32
I16 = mybir.dt.int16
BF_NP = ml_dtypes.bfloat16

P = 128   # partitions / tile node count / chunk edge count
D = 64    # feature dim
DP = 128  # padded feature width of the gather table (256B rows)
GMAX = 8  # max chunks (of 128 rows) per dma_gather instruction (1024 rows)


class Cfg:
    def __init__(self, n_nodes, n_cores, npc_raw, npc, spt, nb, ctb,
                 n_layers=3):
        self.n_nodes = n_nodes
        self.n_cores = n_cores
        self.npc_raw = npc_raw
        self.npc = npc                  # padded nodes per core
        self.tpc = npc // P             # tiles per core
        self.spt = spt                  # tiles per supertile
        self.nst = self.tpc // spt
        self.nb = nb                    # index buckets (core-aligned)
        self.ctb = ctb                  # chunks per (tile, bucket)
        self.cpt = nb * ctb             # chunks per tile
        self.j = spt * self.cpt         # chunks per supertile
        self.ntot = n_cores * npc
        self.bs = self.ntot // nb       # bucket size (rows); must be < 32768
        assert self.bs <= 32768
        assert (n_cores * npc) % nb == 0 and npc * (n_cores // nb) == self.bs
        self.n_layers = n_layers


def _pack_core(sizes, tpc, cap):
    """Assign nodes (rows of `sizes` [n,nb]) to tpc tiles of 128 slots s.t.
    per-tile per-bucket sums <= cap.  Greedy best-fit with a swap-repair
    pass for nodes the greedy can't place.  Returns tile index per node."""
    n, nbk = sizes.shape
    order = np.argsort(-sizes.sum(1), kind="stable")
    rem = np.full((tpc, nbk), cap, np.int64)
    slots = np.full(tpc, P)
    assign = np.full(n, -1, np.int32)
    pending = []
    for i in order:
        s = sizes[i]
        cand = (slots > 0) & np.all(rem >= s, axis=1)
        if not cand.any():
            pending.append(i)
            continue
        scores = (rem - s).min(1).astype(np.float64) + 0.001 * slots
        scores[~cand] = -1e18
        t = int(np.argmax(scores))
        rem[t] -= s
        slots[t] -= 1
        assign[i] = t
    for i in pending:
        s = sizes[i]
        placed = False
        for t in np.argsort(-(rem - s).min(1)):
            in_t = np.where(assign == t)[0]
            ok_j = np.all(rem[t][None, :] + sizes[in_t] - s[None, :] >= 0,
                          axis=1)
            for j in in_t[ok_j]:
                c2 = (slots > 0) & np.all(rem >= sizes[j], axis=1)
                c2[t] = False
                if not c2.any():
                    continue
                sc2 = (rem - sizes[j]).min(1).astype(np.float64)
                sc2[~c2] = -1e18
                t2 = int(np.argmax(sc2))
                rem[t] += sizes[j] - s
                assign[i] = t
                rem[t2] -= sizes[j]
                slots[t2] -= 1
                assign[j] = t2
                placed = True
                break
            if placed:
                break
        if not placed:
            raise RuntimeError("node packing failed; increase ctb")
    assert (assign >= 0).all()
    return assign


def prep_host(x, edge_index, Wl1, cfg: Cfg):
    N = cfg.n_nodes
    NC, NPC_RAW, NPC, TPC, SPT, NB, CTB = (
        cfg.n_cores, cfg.npc_raw, cfg.npc, cfg.tpc, cfg.spt, cfg.nb, cfg.ctb)
    NST, CPT, J = cfg.nst, cfg.cpt, cfg.j
    RCH = SPT * CTB
    BS = cfg.bs
    src = np.asarray(edge_index[0], dtype=np.int64)
    dst = np.asarray(edge_index[1], dtype=np.int64)
    E = src.shape[0]

    deg = np.bincount(src, minlength=N)
    if deg.min() == 0:
        raise NotImplementedError(
            "deg-0 nodes present; the simplified Wg+Ws fusion is invalid")
    inv_deg = (1.0 / deg).astype(np.float32)

    # Buckets are supertile-group slices (NB groups of NST/NB supertiles on
    # every core).  A node's group is fixed a priori from its raw local id,
    # so the gather table can be laid out [group][core][rows-in-group] and
    # the per-layer AllGather splits into NB pipelined slice collectives.
    NGR = NPC_RAW // NB                   # raw nodes per group
    GP = NPC // NB                        # padded positions per group
    TPG = TPC // NB                       # tiles per group
    c_src = np.minimum(src // NPC_RAW, NC - 1)
    c_dst = np.minimum(dst // NPC_RAW, NC - 1)
    loc_raw = np.arange(N) - np.minimum(np.arange(N) // NPC_RAW, NC - 1) \
        * NPC_RAW
    grp = np.minimum(loc_raw // NGR, NB - 1)
    node_core = np.minimum(np.arange(N) // NPC_RAW, NC - 1)

    # The per-(core,group) packing pools are tight (cap 25*384=9600 per
    # bucket vs ~9375 +- 84 demand), so rebalance group membership until
    # every pool-bucket demand has headroom.  Moving a node also relabels
    # its in-edges' buckets, so iterate with fresh counts each round.
    LIMIT = 9520
    for _ in range(30):
        bucket = grp[dst]
        nbcnt = np.zeros((N, NB), np.int32)
        np.add.at(nbcnt, (src, bucket), 1)
        dem = np.zeros((NC, NB, NB), np.int64)      # [core, group, bucket]
        np.add.at(dem, (node_core, grp), nbcnt)
        gsize = np.zeros((NC, NB), np.int64)
        np.add.at(gsize, (node_core, grp), 1)
        over = np.argwhere(dem.max(2) > LIMIT)
        if len(over) == 0:
            break
        for c, g in over:
            b = int(np.argmax(dem[c, g]))
            excess = int(dem[c, g, b] - (LIMIT - 50))
            pool = np.where((node_core == c) & (grp == g))[0]
            cand = pool[np.argsort(-nbcnt[pool, b])]
            moved = 0
            for v in cand[:400]:
                if moved >= excess:
                    break
                # best target: min resulting worst-bucket demand
                res = (dem[c] + nbcnt[v][None, :]).max(1).astype(np.float64)
                res[g] = 1e18
                res[gsize[c] >= NPC // NB] = 1e18
                g2 = int(np.argmin(res))
                if res[g2] > LIMIT - 10:
                    continue
                grp[v] = g2
                dem[c, g] -= nbcnt[v]
                dem[c, g2] += nbcnt[v]
                gsize[c, g] -= 1
                gsize[c, g2] += 1
                moved += nbcnt[v, b]
    else:
        bucket = grp[dst]
        nbcnt = np.zeros((N, NB), np.int32)
        np.add.at(nbcnt, (src, bucket), 1)
        dem = np.zeros((NC, NB, NB), np.int64)
        np.add.at(dem, (node_core, grp), nbcnt)
        if dem.max() > TPC // NB * CTB * P - 10:
            raise RuntimeError(f"group rebalancing plateaued at {dem.max()}")
    bucket = grp[dst]
    nbcnt = np.zeros((N, NB), np.int32)
    np.add.at(nbcnt, (src, bucket), 1)
    perm_pos = np.zeros(N, np.int64)      # orig id -> position within core
    for c in range(NC):
        lo, hi = c * NPC_RAW, min((c + 1) * NPC_RAW, N)
        ids = np.arange(lo, hi)
        for g in range(NB):
            sel = ids[grp[ids] == g]
            n_local = sel.shape[0]
            assign = _pack_core(nbcnt[sel], TPG, CTB * P)
            order_t = np.argsort(assign, kind="stable")
            within = np.arange(n_local) - np.searchsorted(
                assign[order_t], assign[order_t])
            pos = np.empty(n_local, np.int64)
            pos[order_t] = (g * TPG + assign[order_t]) * P + within
            perm_pos[sel] = pos
    gpos = np.minimum(np.arange(N) // NPC_RAW, NC - 1) * NPC + perm_pos

    # table row id in [group][core][pos-in-group] layout
    tbl_row = grp * BS + \
        np.minimum(np.arange(N) // NPC_RAW, NC - 1) * GP + (perm_pos % GP)
    pdst = tbl_row[dst]                   # row in the gather-table layout
    lsrc_tile = (perm_pos[src] % P).astype(np.int16)
    tile_of_src = perm_pos[src] // P      # tile within core
    st_of_src = tile_of_src // SPT
    t_in_st = tile_of_src % SPT

    # slot assignment: group by (core, st, bucket, tile-in-st)
    key = ((c_src * NST + st_of_src) * NB + bucket) * SPT + t_in_st
    n_groups = NC * NST * NB * SPT
    counts = np.bincount(key, minlength=n_groups)
    assert counts.max() <= CTB * P, (counts.max(), CTB * P)
    order = np.argsort(key, kind="stable")
    starts = np.zeros(n_groups + 1, np.int64)
    np.cumsum(counts, out=starts[1:])
    q = np.arange(E) - starts[key[order]]     # position within group
    ks = key[order]
    g_c = ks // (NST * NB * SPT)
    g_st = (ks // (NB * SPT)) % NST
    g_b = (ks // SPT) % NB
    g_t = ks % SPT
    chunk = g_b * RCH + g_t * CTB + q // P   # chunk within supertile
    p = q % P

    lsrc_arr = np.full((NC, NST, P, J), 300, np.int16)
    lsrc_arr[g_c, g_st, p, chunk] = lsrc_tile[order]
    # S depends only on the graph: build the 0/1 indicator on the host and
    # ship it, removing the on-device is_equal/iota and the s_cache store
    s_full = (lsrc_arr[:, :, :, :, None] ==
              np.arange(P, dtype=np.int16)[None, None, None, None, :]
              ).astype(BF_NP).reshape(NC, NST, P, J * P)

    # bucket-relative int16 indices; pads point at bucket row 0 (their
    # contribution is killed by S == 0)
    rel = np.zeros((NC, NST, P, J), np.int16)
    rel[g_c, g_st, p, chunk] = (pdst[order] - g_b * BS).astype(np.int16)

    # dma_gather wrapped layout: idx i (-> partition i%128, chunk i//128 of
    # the output) is read from idxs[i%16, i//16]; replicate over 8 groups.
    W16 = RCH * P // 16                   # 96 idx columns per bucket
    blocks = rel.reshape(NC, NST, P, NB, RCH)
    flat = blocks.transpose(0, 1, 3, 4, 2).reshape(NC, NST, NB, RCH * P)
    w = flat.reshape(NC, NST, NB, W16, 16).transpose(0, 1, 2, 4, 3)
    wfull = np.broadcast_to(w[:, :, :, None, :, :],
                            (NC, NST, NB, 8, 16, W16))
    gidx16 = np.ascontiguousarray(
        wfull.reshape(NC, NST, NB, P, W16).transpose(0, 1, 3, 2, 4)
        .reshape(NC, NST, P, NB * W16))

    # padded per-core node arrays (in permuted order)
    x = np.asarray(x, dtype=np.float32)
    x_pad = np.zeros((cfg.ntot, D), np.float32)
    invdeg_pad = np.zeros(cfg.ntot, np.float32)
    x_pad[gpos] = x
    invdeg_pad[gpos] = inv_deg

    # layer-0 gather table: (x @ Wl1) in bf16, 256B rows, laid out in the
    # [group][core][pos-in-group] order that matches the slice collectives
    t0 = (x @ np.asarray(Wl1, np.float32)).astype(BF_NP)
    table0 = np.zeros((cfg.ntot, DP), BF_NP)
    table0[tbl_row, :D] = t0

    per_core = []
    for c in range(NC):
        xs = x_pad[c * NPC:(c + 1) * NPC]
        per_core.append(dict(
            x_ownT=np.ascontiguousarray(xs.T.astype(BF_NP)),      # [64, NPC]
            table0=table0,                                        # [NTOT, DP]
            gidx16=np.ascontiguousarray(gidx16[c]),               # [NST,128,NB*96]
            s_full=np.ascontiguousarray(s_full[c]),               # [NST,128,J*P]
            invdegT=np.ascontiguousarray(
                invdeg_pad[c * NPC:(c + 1) * NPC].reshape(TPC, P).T),
        ))
    return per_core, perm_pos


def build_program(nc, cfg: Cfg, tc=None):
    NPC, NTOT, NST, SPT, NB, CTB, J, TPC = (
        cfg.npc, cfg.ntot, cfg.nst, cfg.spt, cfg.nb, cfg.ctb, cfg.j, cfg.tpc)
    NL = cfg.n_layers
    RCH = SPT * CTB
    W16 = RCH * P // 16

    x_ownT = nc.dram_tensor("x_ownT", [D, NPC], BF16, kind="ExternalInput")
    table0 = nc.dram_tensor("table0", [NTOT, DP], BF16, kind="ExternalInput")
    gidx16 = nc.dram_tensor("gidx16", [NST, P, NB * W16], I16,
                            kind="ExternalInput")
    s_full = nc.dram_tensor("s_full", [NST, P, J * P], BF16,
                            kind="ExternalInput")
    invdegT = nc.dram_tensor("invdegT", [P, TPC], F32, kind="ExternalInput")
    w_gs = [nc.dram_tensor(f"w_gs{L}", [D, D], BF16, kind="ExternalInput")
            for L in range(NL)]
    w_ln = [nc.dram_tensor(f"w_ln{L}", [D, D], BF16, kind="ExternalInput")
            for L in range(NL - 1)]   # Wl of layer L+1
    out_own = nc.dram_tensor("out_own", [NPC, D], F32, kind="ExternalOutput")

    t2_own = [nc.dram_tensor(f"t2_own{L}", [NPC, DP], BF16, kind="Internal")
              for L in range(NL - 1)]

    h_full = [nc.dram_tensor(f"h_full{L}", [NTOT, DP], BF16, kind="Internal",
                             addr_space="Shared" if cfg.n_cores > 4 else "Local")
              for L in range(NL - 1)]

    own_ctx = tc is None
    if own_ctx:
        tc = tile.TileContext(nc)
        tc.__enter__()
    try:
        _emit(nc, tc, cfg, locals())
    finally:
        if own_ctx:
            tc.__exit__(None, None, None)
    return nc


def _emit(nc, tc, cfg: Cfg, T):
    NPC, NTOT, NST, SPT, NB, CTB, J, TPC, NL = (
        cfg.npc, cfg.ntot, cfg.nst, cfg.spt, cfg.nb, cfg.ctb, cfg.j, cfg.tpc,
        cfg.n_layers)
    x_ownT, table0, gidx16, s_full, invdegT = (
        T["x_ownT"], T["table0"], T["gidx16"], T["s_full"], T["invdegT"])
    w_gs, w_ln, out_own = T["w_gs"], T["w_ln"], T["out_own"]
    t2_own, h_full = T["t2_own"], T["h_full"]
    RCH = SPT * CTB               # chunks per bucket region
    BS = cfg.bs
    W16 = RCH * P // 16

    with (
        tc.tile_pool(name="const", bufs=1) as constp,
        tc.tile_pool(name="io", bufs=4) as iop,
        tc.tile_pool(name="xp", bufs=4) as xp,
        tc.tile_pool(name="big", bufs=3) as bigp,
        tc.tile_pool(name="small", bufs=4) as smallp,
        tc.tile_pool(name="psA", bufs=2, space="PSUM") as psA,
        tc.tile_pool(name="psB", bufs=2, space="PSUM") as psB,
        tc.tile_pool(name="psC", bufs=2, space="PSUM") as psC,
        tc.tile_pool(name="psD", bufs=2, space="PSUM") as psD,
    ):
        ident = constp.tile([P, P], BF16, name="ident")
        make_identity(nc, ident[:])
        invdeg_sb = constp.tile([P, TPC], F32, name="invdeg_sb")
        nc.sync.dma_start(invdeg_sb[:], invdegT[:])
        wgs_sb, wln_sb = [], []
        for L in range(NL):
            wg_t = constp.tile([D, D], BF16, name=f"wgs_sb{L}")
            nc.sync.dma_start(wg_t[:], w_gs[L][:])
            wgs_sb.append(wg_t)
        for L in range(NL - 1):
            wl_t = constp.tile([D, D], BF16, name=f"wln_sb{L}")
            nc.sync.dma_start(wl_t[:], w_ln[L][:])
            wln_sb.append(wl_t)
        hT = [constp.tile([D, NPC], BF16, name=f"hT{i}") for i in range(2)]
        nc.sync.dma_start(hT[0][:], x_ownT[:])

        def emit_collective(L, g):
            GPp = NPC // NB
            nc.gpsimd.collective_compute(
                "AllGather",
                mybir.AluOpType.bypass,
                replica_groups=[list(range(cfg.n_cores))],
                ins=[t2_own[L][g * GPp:(g + 1) * GPp, :]],
                outs=[h_full[L][g * BS:(g + 1) * BS, :]],
            )

        for L in range(NL):
            table = table0 if L == 0 else h_full[L - 1]
            hT_in, hT_out = hT[L % 2], hT[(L + 1) % 2]
            last = L == NL - 1
            # Queue pattern pairs each Q7 core pair with one 1024-idx and
            # one 896-idx gather per supertile (plain round-robin gives two
            # 1024s to q0/q2).
            QPAT = [0, 1, 2, 3, 1, 0, 3, 2]

            def emit_idx(s):
                idx_t = iop.tile([P, NB * W16], I16, tag="idx",
                                 name=f"idx_{L}_{s}")
                nc.sync.dma_start(idx_t[:], gidx16[s, :, :])
                X = xp.tile([P, J * DP], BF16, tag="X", name=f"X_{L}_{s}")
                return idx_t, X, X[:].rearrange("p (c e) -> p c e", e=DP)

            def emit_gathers(st, buckets, gq0):
                # <= 1024 rows (8 chunks) per dma_gather (HW limit); each
                # queue_num runs on its own Q7 core pair, 4 run concurrently
                idx_t, X, X3 = st
                gq = gq0
                for b in buckets:
                    for c0 in range(0, RCH, GMAX):
                        c1 = min(c0 + GMAX, RCH)
                        nidx = (c1 - c0) * P
                        nc.gpsimd.dma_gather(
                            out_ap=X3[:, b * RCH + c0:b * RCH + c1, :],
                            in_ap=table[b * BS:(b + 1) * BS, :],
                            idxs_ap=idx_t[:, b * W16 + c0 * (P // 16):
                                          b * W16 + c1 * (P // 16)],
                            num_idxs=nidx,
                            num_idxs_reg=nidx,
                            elem_size=DP,
                            queue_num=QPAT[gq % len(QPAT)],
                        )
                        gq += 1

            # For layers that read a freshly AllGathered table, interleave
            # the first two supertiles' gathers so buckets 0-2 (whose slice
            # collectives finished earlier) fill the GPSIMD stream while the
            # last group's collective completes; only the b3 gathers wait.
            pre_emitted = {}
            if L > 0:
                st0, st1 = emit_idx(0), emit_idx(1)
                emit_gathers(st0, range(NB - 1), 0)
                emit_gathers(st1, range(NB - 1), 0)
                emit_gathers(st0, [NB - 1], 2 * (NB - 1))
                emit_gathers(st1, [NB - 1], 2 * (NB - 1))
                pre_emitted = {0: st0, 1: st1}

            pending_cc = None
            for s in range(NST):
                if s in pre_emitted:
                    idx_t, X, X3 = pre_emitted[s]
                else:
                    st = emit_idx(s)
                    emit_gathers(st, range(NB), 0)
                    idx_t, X, X3 = st


                S = bigp.tile([P, J * P], BF16, tag="S", name=f"S_{L}_{s}")
                nc.sync.dma_start(S[:], s_full[s, :, :])

                pre_st = bigp.tile([P, SPT * D], F32, tag="pre",
                                   name=f"pre_{L}_{s}")
                if last:
                    hnew = bigp.tile([P, SPT * D], F32, tag="hnew",
                                     name=f"hn_{L}_{s}")
                else:
                    hnb = bigp.tile([P, SPT * D], BF16, tag="hnb",
                                    name=f"hb_{L}_{s}")
                    t2b = bigp.tile([P, SPT * DP], BF16, tag="t2b",
                                    name=f"t2_{L}_{s}")

                for t in range(SPT):
                    g_t = s * SPT + t
                    chunks = [b * RCH + t * CTB + k
                              for b in range(NB) for k in range(CTB)]
                    SU = psA.tile([P, D], F32, tag="SU", name=f"SU_{L}_{s}_{t}")
                    for ci, ch in enumerate(chunks):
                        nc.tensor.matmul(
                            SU[:],
                            lhsT=S[:, ch * P:(ch + 1) * P],
                            rhs=X3[:, ch, 0:D],
                            start=(ci == 0), stop=(ci == len(chunks) - 1),
                        )
                    gP = psC.tile([P, D], F32, tag="gP", name=f"gP_{L}_{s}_{t}")
                    nc.tensor.matmul(
                        gP[:], lhsT=hT_in[:, g_t * P:(g_t + 1) * P],
                        rhs=wgs_sb[L][:], start=True, stop=True)
                    # pre = SU * invdeg + g  (two ops: only one PSUM input
                    # allowed per DVE instruction)
                    e_sb = smallp.tile([P, D], F32, tag="e",
                                       name=f"e_{L}_{s}_{t}")
                    nc.vector.tensor_scalar_mul(
                        e_sb[:], SU[:], invdeg_sb[:, g_t:g_t + 1])
                    nc.vector.tensor_add(
                        pre_st[:, t * D:(t + 1) * D], e_sb[:], gP[:])

                # batched ELU over the whole supertile:
                # out = (max(x,0)-1) + exp(min(x,0))
                lo = bigp.tile([P, SPT * D], F32, tag="lo", name=f"lo_{L}_{s}")
                nc.vector.tensor_scalar_min(lo[:], pre_st[:], 0.0)
                ex = bigp.tile([P, SPT * D], F32, tag="ex", name=f"ex_{L}_{s}")
                nc.scalar.activation(ex[:], lo[:],
                                     mybir.ActivationFunctionType.Exp)
                hi1 = bigp.tile([P, SPT * D], F32, tag="hi1",
                                name=f"hi_{L}_{s}")
                nc.vector.tensor_scalar(
                    hi1[:], pre_st[:], 0.0, 1.0,
                    op0=mybir.AluOpType.max, op1=mybir.AluOpType.subtract)
                nc.vector.tensor_add(hnew[:] if last else hnb[:],
                                     ex[:], hi1[:])

                if not last:
                    for t in range(SPT):
                        g_t = s * SPT + t
                        hTP = psD.tile([D, P], BF16, tag="hTP",
                                       name=f"hTP_{L}_{s}_{t}")
                        nc.tensor.transpose(
                            hTP[:], hnb[:, t * D:(t + 1) * D], ident[:])
                        nc.vector.tensor_copy(
                            hT_out[:, g_t * P:(g_t + 1) * P], hTP[:])
                        t2P = psB.tile([P, D], F32, tag="t2P",
                                       name=f"t2P_{L}_{s}_{t}")
                        nc.tensor.matmul(
                            t2P[:], lhsT=hT_out[:, g_t * P:(g_t + 1) * P],
                            rhs=wln_sb[L][:], start=True, stop=True)
                        nc.vector.tensor_copy(
                            t2b[:, t * DP:t * DP + D], t2P[:])

                if last:
                    dst_rows = out_own.rearrange(
                        "(s t p) d -> s p t d", s=NST, t=SPT, p=P)
                    nc.sync.dma_start(
                        dst_rows[s],
                        hnew[:].rearrange("p (t d) -> p t d", d=D))
                else:
                    t2_rows = t2_own[L].rearrange(
                        "(s t p) d -> s p t d", s=NST, t=SPT, p=P)
                    nc.sync.dma_start(
                        t2_rows[s],
                        t2b[:].rearrange("p (t d) -> p t d", d=DP))
                    # supertile groups map 1:1 to table slices: fire the
                    # slice collective as soon as its group is done so the
                    # transfer overlaps the remaining groups' compute
                    if (s + 1) % (NST // NB) == 0:
                        emit_collective(L, s // (NST // NB))


def _make_cfg_full():
    return Cfg(n_nodes=100000, n_cores=8, npc_raw=12500, npc=12800,
               spt=5, nb=4, ctb=3)


def kernel(**inputs):
    cfg = _make_cfg_full()
    x = np.asarray(inputs["x"], np.float32)
    ei = np.asarray(inputs["edge_index"])
    Wgs, Wl = [], []
    for L, (a, b, c, bias) in enumerate(
            [("Wg1", "Wl1", "Ws1", "b1"), ("Wg2", "Wl2", "Ws2", "b2"),
             ("Wgo", "Wlo", "Wso", "bo")]):
        bv = np.asarray(inputs[bias], np.float32)
        assert np.all(bv == 0.0), "nonzero bias not supported by this build"
        Wgs.append((np.asarray(inputs[a], np.float32) +
                    np.asarray(inputs[c], np.float32)).astype(BF_NP))
        Wl.append(np.asarray(inputs[b], np.float32))

    per_core, perm_pos = prep_host(x, ei, Wl[0], cfg)

    nc = bacc.Bacc("TRN2", target_bir_lowering=False, debug=False,
                   enable_asserts=False, num_devices=cfg.n_cores,
                   num_swdge_queues=4)
    build_program(nc, cfg)
    nc.compile()

    in_maps = []
    for c in range(cfg.n_cores):
        m = dict(per_core[c])
        for L in range(3):
            m[f"w_gs{L}"] = Wgs[L]
        for L in range(2):
            m[f"w_ln{L}"] = Wl[L + 1].astype(BF_NP)
        in_maps.append(m)

    res = run_bass_kernel_spmd(
        nc, in_maps, core_ids=list(range(cfg.n_cores)),
        trace=bool(int(os.environ.get("GNN_TRACE", "0"))),
    )
    full = np.zeros((cfg.n_nodes, D), np.float32)
    for c in range(cfg.n_cores):
        lo = c * cfg.npc_raw
        hi = min((c + 1) * cfg.npc_raw, cfg.n_nodes)
        full[lo:hi] = res.results[c]["out_own"][perm_pos[lo:hi]]
    kernel.last_results = res
    return full.astype(np.float32)


# revision 40
# speedup vs baseline: 1.1021x; 1.1021x over previous
"""DEMONet 3-layer GNN message-passing kernel for 8x Trainium2 NeuronCores.

Math per layer (verified against reference; all nodes have deg > 0):
    nm   = segment_sum(h[dst], src) / deg
    out  = elu(h @ (Wg + Ws) + nm @ Wl + b)          (b == 0 here)

Final design (5.60ms baseline -> 1.38ms):
  * Nodes row-partitioned across 8 cores (12800 padded/core); edges live
    with their src node's core; supertiles of SPT=5 tiles (128 nodes each).
  * Gather table for layer L holds rows (h_{L-1} @ Wl_L) in bf16 padded to
    128 cols (256B rows, pad never read), so the on-device segment-sum
    directly yields nm @ Wl.  Layer 0's table (x @ Wl1) comes from the host.
  * Neighbour rows fetched with gpsimd.dma_gather, <=1024 rows per call
    (hard HW limit), int16 indices relative to one of NB=4 bucket windows
    (25600 rows < 32768).  queue_num selects the Q7 core pair that runs the
    descgen ucode: 4 queues run concurrently (4x), with a pairing pattern
    that gives every queue one 1024- and one 896-idx gather per supertile.
  * Buckets are supertile-group slices of the table ([group][core][rows]
    layout, iteratively rebalanced group membership to keep the packing
    pools feasible), so each layer's AllGather splits into 4 slice
    collectives that fire as their group completes and overlap compute.
  * Segment-sum on the TensorEngine: S_ch [128e,128n] (0/1, built on the
    HOST, shipped as input) x X_ch [128e,64] accumulate PSUM per tile.
  * h kept transposed [64, NPC] bf16 in SBUF across layers; per tile:
    g-term matmul, (SU*invdeg)+g, supertile-batched ELU, transpose to
    update hT, and next-layer table-row matmul.
"""

import os
import numpy as np
import ml_dtypes

import concourse.bass as bass
import concourse.bacc as bacc
import concourse.mybir as mybir
import concourse.tile as tile
from concourse.bass_utils import run_bass_kernel_spmd
from concourse.masks import make_identity

F32 = mybir.dt.float32
BF16 = mybir.dt.bfloat16
I32 = mybir.dt.int32
I16 = mybir.dt.int16
BF_NP = ml_dtypes.bfloat16

P = 128   # partitions / tile node count / chunk edge count
D = 64    # feature dim
DP = 128  # padded feature width of the gather table (256B rows)
GMAX = 8  # max chunks (of 128 rows) per dma_gather instruction (1024 rows)


class Cfg:
    def __init__(self, n_nodes, n_cores, npc_raw, npc, spt, nb, ctb,
                 n_layers=3):
        self.n_nodes = n_nodes
        self.n_cores = n_cores
        self.npc_raw = npc_raw
        self.npc = npc                  # padded nodes per core
        self.tpc = npc // P             # tiles per core
        self.spt = spt                  # tiles per supertile
        self.nst = self.tpc // spt
        self.nb = nb                    # index buckets (core-aligned)
        self.ctb = ctb                  # chunks per (tile, bucket)
        self.cpt = nb * ctb             # chunks per tile
        self.j = spt * self.cpt         # chunks per supertile
        self.ntot = n_cores * npc
        self.bs = self.ntot // nb       # bucket size (rows); must be < 32768
        assert self.bs <= 32768
        assert (n_cores * npc) % nb == 0 and npc * (n_cores // nb) == self.bs
        self.n_layers = n_layers


def _pack_core(sizes, tpc, cap):
    """Assign nodes (rows of `sizes` [n,nb]) to tpc tiles of 128 slots s.t.
    per-tile per-bucket sums <= cap.  Greedy best-fit with a swap-repair
    pass for nodes the greedy can't place.  Returns tile index per node."""
    n, nbk = sizes.shape
    order = np.argsort(-sizes.sum(1), kind="stable")
    rem = np.full((tpc, nbk), cap, np.int64)
    slots = np.full(tpc, P)
    assign = np.full(n, -1, np.int32)
    pending = []
    for i in order:
        s = sizes[i]
        cand = (slots > 0) & np.all(rem >= s, axis=1)
        if not cand.any():
            pending.append(i)
            continue
        scores = (rem - s).min(1).astype(np.float64) + 0.001 * slots
        scores[~cand] = -1e18
        t = int(np.argmax(scores))
        rem[t] -= s
        slots[t] -= 1
        assign[i] = t
    for i in pending:
        s = sizes[i]
        placed = False
        for t in np.argsort(-(rem - s).min(1)):
            in_t = np.where(assign == t)[0]
            ok_j = np.all(rem[t][None, :] + sizes[in_t] - s[None, :] >= 0,
                          axis=1)
            for j in in_t[ok_j]:
                c2 = (slots > 0) & np.all(rem >= sizes[j], axis=1)
                c2[t] = False
                if not c2.any():
                    continue
                sc2 = (rem - sizes[j]).min(1).astype(np.float64)
                sc2[~c2] = -1e18
                t2 = int(np.argmax(sc2))
                rem[t] += sizes[j] - s
                assign[i] = t
                rem[t2] -= sizes[j]
                slots[t2] -= 1
                assign[j] = t2
                placed = True
                break
            if placed:
                break
        if not placed:
            raise RuntimeError("node packing failed; increase ctb")
    assert (assign >= 0).all()
    return assign


def prep_host(x, edge_index, Wl1, cfg: Cfg):
    N = cfg.n_nodes
    NC, NPC_RAW, NPC, TPC, SPT, NB, CTB = (
        cfg.n_cores, cfg.npc_raw, cfg.npc, cfg.tpc, cfg.spt, cfg.nb, cfg.ctb)
    NST, CPT, J = cfg.nst, cfg.cpt, cfg.j
    RCH = SPT * CTB
    BS = cfg.bs
    src = np.asarray(edge_index[0], dtype=np.int64)
    dst = np.asarray(edge_index[1], dtype=np.int64)
    E = src.shape[0]

    deg = np.bincount(src, minlength=N)
    if deg.min() == 0:
        raise NotImplementedError(
            "deg-0 nodes present; the simplified Wg+Ws fusion is invalid")
    inv_deg = (1.0 / deg).astype(np.float32)

    # Buckets are supertile-group slices (NB groups of NST/NB supertiles on
    # every core).  A node's group is fixed a priori from its raw local id,
    # so the gather table can be laid out [group][core][rows-in-group] and
    # the per-layer AllGather splits into NB pipelined slice collectives.
    NGR = NPC_RAW // NB                   # raw nodes per group
    GP = NPC // NB                        # padded positions per group
    TPG = TPC // NB                       # tiles per group
    c_src = np.minimum(src // NPC_RAW, NC - 1)
    c_dst = np.minimum(dst // NPC_RAW, NC - 1)
    loc_raw = np.arange(N) - np.minimum(np.arange(N) // NPC_RAW, NC - 1) \
        * NPC_RAW
    grp = np.minimum(loc_raw // NGR, NB - 1)
    node_core = np.minimum(np.arange(N) // NPC_RAW, NC - 1)

    # The per-(core,group) packing pools are tight (cap 25*384=9600 per
    # bucket vs ~9375 +- 84 demand), so rebalance group membership until
    # every pool-bucket demand has headroom.  Moving a node also relabels
    # its in-edges' buckets, so iterate with fresh counts each round.
    LIMIT = 9520
    for _ in range(30):
        bucket = grp[dst]
        nbcnt = np.zeros((N, NB), np.int32)
        np.add.at(nbcnt, (src, bucket), 1)
        dem = np.zeros((NC, NB, NB), np.int64)      # [core, group, bucket]
        np.add.at(dem, (node_core, grp), nbcnt)
        gsize = np.zeros((NC, NB), np.int64)
        np.add.at(gsize, (node_core, grp), 1)
        over = np.argwhere(dem.max(2) > LIMIT)
        if len(over) == 0:
            break
        for c, g in over:
            b = int(np.argmax(dem[c, g]))
            excess = int(dem[c, g, b] - (LIMIT - 50))
            pool = np.where((node_core == c) & (grp == g))[0]
            cand = pool[np.argsort(-nbcnt[pool, b])]
            moved = 0
            for v in cand[:400]:
                if moved >= excess:
                    break
                # best target: min resulting worst-bucket demand
                res = (dem[c] + nbcnt[v][None, :]).max(1).astype(np.float64)
                res[g] = 1e18
                res[gsize[c] >= NPC // NB] = 1e18
                g2 = int(np.argmin(res))
                if res[g2] > LIMIT - 10:
                    continue
                grp[v] = g2
                dem[c, g] -= nbcnt[v]
                dem[c, g2] += nbcnt[v]
                gsize[c, g] -= 1
                gsize[c, g2] += 1
                moved += nbcnt[v, b]
    else:
        bucket = grp[dst]
        nbcnt = np.zeros((N, NB), np.int32)
        np.add.at(nbcnt, (src, bucket), 1)
        dem = np.zeros((NC, NB, NB), np.int64)
        np.add.at(dem, (node_core, grp), nbcnt)
        if dem.max() > TPC // NB * CTB * P - 10:
            raise RuntimeError(f"group rebalancing plateaued at {dem.max()}")
    bucket = grp[dst]
    nbcnt = np.zeros((N, NB), np.int32)
    np.add.at(nbcnt, (src, bucket), 1)
    perm_pos = np.zeros(N, np.int64)      # orig id -> position within core
    for c in range(NC):
        lo, hi = c * NPC_RAW, min((c + 1) * NPC_RAW, N)
        ids = np.arange(lo, hi)
        for g in range(NB):
            sel = ids[grp[ids] == g]
            n_local = sel.shape[0]
            assign = _pack_core(nbcnt[sel], TPG, CTB * P)
            order_t = np.argsort(assign, kind="stable")
            within = np.arange(n_local) - np.searchsorted(
                assign[order_t], assign[order_t])
            pos = np.empty(n_local, np.int64)
            pos[order_t] = (g * TPG + assign[order_t]) * P + within
            perm_pos[sel] = pos
    gpos = np.minimum(np.arange(N) // NPC_RAW, NC - 1) * NPC + perm_pos

    # table row id in [group][core][pos-in-group] layout
    tbl_row = grp * BS + \
        np.minimum(np.arange(N) // NPC_RAW, NC - 1) * GP + (perm_pos % GP)
    pdst = tbl_row[dst]                   # row in the gather-table layout
    lsrc_tile = (perm_pos[src] % P).astype(np.int16)
    tile_of_src = perm_pos[src] // P      # tile within core
    st_of_src = tile_of_src // SPT
    t_in_st = tile_of_src % SPT

    # slot assignment: group by (core, st, bucket, tile-in-st)
    key = ((c_src * NST + st_of_src) * NB + bucket) * SPT + t_in_st
    n_groups = NC * NST * NB * SPT
    counts = np.bincount(key, minlength=n_groups)
    assert counts.max() <= CTB * P, (counts.max(), CTB * P)
    order = np.argsort(key, kind="stable")
    starts = np.zeros(n_groups + 1, np.int64)
    np.cumsum(counts, out=starts[1:])
    q = np.arange(E) - starts[key[order]]     # position within group
    ks = key[order]
    g_c = ks // (NST * NB * SPT)
    g_st = (ks // (NB * SPT)) % NST
    g_b = (ks // SPT) % NB
    g_t = ks % SPT
    chunk = g_b * RCH + g_t * CTB + q // P   # chunk within supertile
    p = q % P

    lsrc_arr = np.full((NC, NST, P, J), 300, np.int16)
    lsrc_arr[g_c, g_st, p, chunk] = lsrc_tile[order]
    # S depends only on the graph: build the 0/1 indicator on the host and
    # ship it, removing the on-device is_equal/iota and the s_cache store
    s_full = (lsrc_arr[:, :, :, :, None] ==
              np.arange(P, dtype=np.int16)[None, None, None, None, :]
              ).astype(BF_NP).reshape(NC, NST, P, J * P)

    # bucket-relative int16 indices; pads point at bucket row 0 (their
    # contribution is killed by S == 0)
    rel = np.zeros((NC, NST, P, J), np.int16)
    rel[g_c, g_st, p, chunk] = (pdst[order] - g_b * BS).astype(np.int16)

    # dma_gather wrapped layout: idx i (-> partition i%128, chunk i//128 of
    # the output) is read from idxs[i%16, i//16]; replicate over 8 groups.
    W16 = RCH * P // 16                   # 96 idx columns per bucket
    blocks = rel.reshape(NC, NST, P, NB, RCH)
    flat = blocks.transpose(0, 1, 3, 4, 2).reshape(NC, NST, NB, RCH * P)
    w = flat.reshape(NC, NST, NB, W16, 16).transpose(0, 1, 2, 4, 3)
    wfull = np.broadcast_to(w[:, :, :, None, :, :],
                            (NC, NST, NB, 8, 16, W16))
    gidx16 = np.ascontiguousarray(
        wfull.reshape(NC, NST, NB, P, W16).transpose(0, 1, 3, 2, 4)
        .reshape(NC, NST, P, NB * W16))

    # padded per-core node arrays (in permuted order)
    x = np.asarray(x, dtype=np.float32)
    x_pad = np.zeros((cfg.ntot, D), np.float32)
    invdeg_pad = np.zeros(cfg.ntot, np.float32)
    x_pad[gpos] = x
    invdeg_pad[gpos] = inv_deg

    # layer-0 gather table: (x @ Wl1) in bf16, 256B rows, laid out in the
    # [group][core][pos-in-group] order that matches the slice collectives
    t0 = (x @ np.asarray(Wl1, np.float32)).astype(BF_NP)
    table0 = np.zeros((cfg.ntot, DP), BF_NP)
    table0[tbl_row, :D] = t0

    per_core = []
    for c in range(NC):
        xs = x_pad[c * NPC:(c + 1) * NPC]
        per_core.append(dict(
            x_ownT=np.ascontiguousarray(xs.T.astype(BF_NP)),      # [64, NPC]
            table0=table0,                                        # [NTOT, DP]
            gidx16=np.ascontiguousarray(gidx16[c]),               # [NST,128,NB*96]
            s_full=np.ascontiguousarray(s_full[c]),               # [NST,128,J*P]
            invdegT=np.ascontiguousarray(
                invdeg_pad[c * NPC:(c + 1) * NPC].reshape(TPC, P).T),
        ))
    return per_core, perm_pos


def build_program(nc, cfg: Cfg, tc=None):
    NPC, NTOT, NST, SPT, NB, CTB, J, TPC = (
        cfg.npc, cfg.ntot, cfg.nst, cfg.spt, cfg.nb, cfg.ctb, cfg.j, cfg.tpc)
    NL = cfg.n_layers
    RCH = SPT * CTB
    W16 = RCH * P // 16

    x_ownT = nc.dram_tensor("x_ownT", [D, NPC], BF16, kind="ExternalInput")
    table0 = nc.dram_tensor("table0", [NTOT, DP], BF16, kind="ExternalInput")
    gidx16 = nc.dram_tensor("gidx16", [NST, P, NB * W16], I16,
                            kind="ExternalInput")
    s_full = nc.dram_tensor("s_full", [NST, P, J * P], BF16,
                            kind="ExternalInput")
    invdegT = nc.dram_tensor("invdegT", [P, TPC], F32, kind="ExternalInput")
    w_gs = [nc.dram_tensor(f"w_gs{L}", [D, D], BF16, kind="ExternalInput")
            for L in range(NL)]
    w_ln = [nc.dram_tensor(f"w_ln{L}", [D, D], BF16, kind="ExternalInput")
            for L in range(NL - 1)]   # Wl of layer L+1
    out_own = nc.dram_tensor("out_own", [NPC, D], F32, kind="ExternalOutput")

    t2_own = [nc.dram_tensor(f"t2_own{L}", [NPC, DP], BF16, kind="Internal")
              for L in range(NL - 1)]

    h_full = [nc.dram_tensor(f"h_full{L}", [NTOT, DP], BF16, kind="Internal",
                             addr_space="Shared" if cfg.n_cores > 4 else "Local")
              for L in range(NL - 1)]

    own_ctx = tc is None
    if own_ctx:
        tc = tile.TileContext(nc)
        tc.__enter__()
    try:
        _emit(nc, tc, cfg, locals())
    finally:
        if own_ctx:
            tc.__exit__(None, None, None)
    return nc


def _emit(nc, tc, cfg: Cfg, T):
    NPC, NTOT, NST, SPT, NB, CTB, J, TPC, NL = (
        cfg.npc, cfg.ntot, cfg.nst, cfg.spt, cfg.nb, cfg.ctb, cfg.j, cfg.tpc,
        cfg.n_layers)
    x_ownT, table0, gidx16, s_full, invdegT = (
        T["x_ownT"], T["table0"], T["gidx16"], T["s_full"], T["invdegT"])
    w_gs, w_ln, out_own = T["w_gs"], T["w_ln"], T["out_own"]
    t2_own, h_full = T["t2_own"], T["h_full"]
    RCH = SPT * CTB               # chunks per bucket region
    BS = cfg.bs
    W16 = RCH * P // 16

    with (
        tc.tile_pool(name="const", bufs=1) as constp,
        tc.tile_pool(name="io", bufs=3) as iop,
        tc.tile_pool(name="xp", bufs=3) as xp,
        tc.tile_pool(name="big", bufs=2) as bigp,
        tc.tile_pool(name="small", bufs=4) as smallp,
        tc.tile_pool(name="psA", bufs=2, space="PSUM") as psA,
        tc.tile_pool(name="psB", bufs=2, space="PSUM") as psB,
        tc.tile_pool(name="psC", bufs=2, space="PSUM") as psC,
        tc.tile_pool(name="psD", bufs=2, space="PSUM") as psD,
    ):
        ident = constp.tile([P, P], BF16, name="ident")
        make_identity(nc, ident[:])

        # Queue pattern pairs each Q7 core pair with one 1024-idx and one
        # 896-idx gather per supertile.
        QPAT = [0, 1, 2, 3, 1, 0, 3, 2]

        def emit_idx(L, s):
            idx_t = iop.tile([P, NB * W16], I16, tag="idx",
                             name=f"idx_{L}_{s}")
            nc.sync.dma_start(idx_t[:], gidx16[s, :, :])
            X = xp.tile([P, J * DP], BF16, tag="X", name=f"X_{L}_{s}")
            return idx_t, X, X[:].rearrange("p (c e) -> p c e", e=DP)

        def emit_gathers(table, st, buckets, gq0):
            # <= 1024 rows (8 chunks) per dma_gather (HW limit); each
            # queue_num runs on its own Q7 core pair, 4 run concurrently
            idx_t, X, X3 = st
            gq = gq0
            for b in buckets:
                for c0 in range(0, RCH, GMAX):
                    c1 = min(c0 + GMAX, RCH)
                    nidx = (c1 - c0) * P
                    nc.gpsimd.dma_gather(
                        out_ap=X3[:, b * RCH + c0:b * RCH + c1, :],
                        in_ap=table[b * BS:(b + 1) * BS, :],
                        idxs_ap=idx_t[:, b * W16 + c0 * (P // 16):
                                      b * W16 + c1 * (P // 16)],
                        num_idxs=nidx,
                        num_idxs_reg=nidx,
                        elem_size=DP,
                        queue_num=QPAT[gq % len(QPAT)],
                    )
                    gq += 1

        # warm start: layer 0's first supertiles' gathers depend only on
        # their idx rows and table0 (an input) — emit them before the
        # constant loads so the gather stream starts immediately
        warm = {}
        for s in (0, 1):
            st = emit_idx(0, s)
            emit_gathers(table0, st, range(NB), 0)
            warm[s] = st

        invdeg_sb = constp.tile([P, TPC], F32, name="invdeg_sb")
        nc.sync.dma_start(invdeg_sb[:], invdegT[:])
        wgs_sb, wln_sb = [], []
        for L in range(NL):
            wg_t = constp.tile([D, D], BF16, name=f"wgs_sb{L}")
            nc.sync.dma_start(wg_t[:], w_gs[L][:])
            wgs_sb.append(wg_t)
        for L in range(NL - 1):
            wl_t = constp.tile([D, D], BF16, name=f"wln_sb{L}")
            nc.sync.dma_start(wl_t[:], w_ln[L][:])
            wln_sb.append(wl_t)
        hT = [constp.tile([D, NPC], BF16, name=f"hT{i}") for i in range(2)]
        nc.sync.dma_start(hT[0][:], x_ownT[:])

        def emit_collective(L, g):
            GPp = NPC // NB
            nc.gpsimd.collective_compute(
                "AllGather",
                mybir.AluOpType.bypass,
                replica_groups=[list(range(cfg.n_cores))],
                ins=[t2_own[L][g * GPp:(g + 1) * GPp, :]],
                outs=[h_full[L][g * BS:(g + 1) * BS, :]],
            )

        for L in range(NL):
            table = table0 if L == 0 else h_full[L - 1]
            hT_in, hT_out = hT[L % 2], hT[(L + 1) % 2]
            last = L == NL - 1
            # For layers that read a freshly AllGathered table, interleave
            # the first two supertiles' gathers so buckets 0-2 (whose slice
            # collectives finished earlier) fill the GPSIMD stream while the
            # last group's collective completes; only the b3 gathers wait.
            # Layer 0's first supertiles were warm-started above.
            if L == 0:
                pre_emitted = warm
            else:
                st0, st1 = emit_idx(L, 0), emit_idx(L, 1)
                emit_gathers(table, st0, range(NB - 1), 0)
                emit_gathers(table, st1, range(NB - 1), 0)
                emit_gathers(table, st0, [NB - 1], 2 * (NB - 1))
                emit_gathers(table, st1, [NB - 1], 2 * (NB - 1))
                pre_emitted = {0: st0, 1: st1}

            pending_cc = None
            for s in range(NST):
                if s in pre_emitted:
                    idx_t, X, X3 = pre_emitted[s]
                else:
                    st = emit_idx(L, s)
                    emit_gathers(table, st, range(NB), 0)
                    idx_t, X, X3 = st


                S = bigp.tile([P, J * P], BF16, tag="S", name=f"S_{L}_{s}")
                nc.sync.dma_start(S[:], s_full[s, :, :])

                pre_st = bigp.tile([P, SPT * D], F32, tag="pre",
                                   name=f"pre_{L}_{s}")
                if last:
                    hnew = bigp.tile([P, SPT * D], F32, tag="hnew",
                                     name=f"hn_{L}_{s}")
                else:
                    hnb = bigp.tile([P, SPT * D], BF16, tag="hnb",
                                    name=f"hb_{L}_{s}")
                    t2b = bigp.tile([P, SPT * DP], BF16, tag="t2b",
                                    name=f"t2_{L}_{s}")

                for t in range(SPT):
                    g_t = s * SPT + t
                    chunks = [b * RCH + t * CTB + k
                              for b in range(NB) for k in range(CTB)]
                    SU = psA.tile([P, D], F32, tag="SU", name=f"SU_{L}_{s}_{t}")
                    for ci, ch in enumerate(chunks):
                        nc.tensor.matmul(
                            SU[:],
                            lhsT=S[:, ch * P:(ch + 1) * P],
                            rhs=X3[:, ch, 0:D],
                            start=(ci == 0), stop=(ci == len(chunks) - 1),
                        )
                    gP = psC.tile([P, D], F32, tag="gP", name=f"gP_{L}_{s}_{t}")
                    nc.tensor.matmul(
                        gP[:], lhsT=hT_in[:, g_t * P:(g_t + 1) * P],
                        rhs=wgs_sb[L][:], start=True, stop=True)
                    # pre = SU * invdeg + g  (two ops: only one PSUM input
                    # allowed per DVE instruction)
                    e_sb = smallp.tile([P, D], F32, tag="e",
                                       name=f"e_{L}_{s}_{t}")
                    nc.vector.tensor_scalar_mul(
                        e_sb[:], SU[:], invdeg_sb[:, g_t:g_t + 1])
                    nc.vector.tensor_add(
                        pre_st[:, t * D:(t + 1) * D], e_sb[:], gP[:])

                # batched ELU over the whole supertile:
                # out = (max(x,0)-1) + exp(min(x,0))
                lo = bigp.tile([P, SPT * D], F32, tag="lo", name=f"lo_{L}_{s}")
                nc.vector.tensor_scalar_min(lo[:], pre_st[:], 0.0)
                ex = bigp.tile([P, SPT * D], F32, tag="ex", name=f"ex_{L}_{s}")
                nc.scalar.activation(ex[:], lo[:],
                                     mybir.ActivationFunctionType.Exp)
                hi1 = bigp.tile([P, SPT * D], F32, tag="hi1",
                                name=f"hi_{L}_{s}")
                nc.vector.tensor_scalar(
                    hi1[:], pre_st[:], 0.0, 1.0,
                    op0=mybir.AluOpType.max, op1=mybir.AluOpType.subtract)
                nc.vector.tensor_add(hnew[:] if last else hnb[:],
                                     ex[:], hi1[:])

                if not last:
                    for t in range(SPT):
                        g_t = s * SPT + t
                        hTP = psD.tile([D, P], BF16, tag="hTP",
                                       name=f"hTP_{L}_{s}_{t}")
                        nc.tensor.transpose(
                            hTP[:], hnb[:, t * D:(t + 1) * D], ident[:])
                        nc.vector.tensor_copy(
                            hT_out[:, g_t * P:(g_t + 1) * P], hTP[:])
                        t2P = psB.tile([P, D], F32, tag="t2P",
                                       name=f"t2P_{L}_{s}_{t}")
                        nc.tensor.matmul(
                            t2P[:], lhsT=hT_out[:, g_t * P:(g_t + 1) * P],
                            rhs=wln_sb[L][:], start=True, stop=True)
                        nc.vector.tensor_copy(
                            t2b[:, t * DP:t * DP + D], t2P[:])

                if last:
                    dst_rows = out_own.rearrange(
                        "(s t p) d -> s p t d", s=NST, t=SPT, p=P)
                    nc.sync.dma_start(
                        dst_rows[s],
                        hnew[:].rearrange("p (t d) -> p t d", d=D))
                else:
                    t2_rows = t2_own[L].rearrange(
                        "(s t p) d -> s p t d", s=NST, t=SPT, p=P)
                    nc.sync.dma_start(
                        t2_rows[s],
                        t2b[:].rearrange("p (t d) -> p t d", d=DP))
                    # supertile groups map 1:1 to table slices: fire the
                    # slice collective as soon as its group is done so the
                    # transfer overlaps the remaining groups' compute
                    if (s + 1) % (NST // NB) == 0:
                        emit_collective(L, s // (NST // NB))


def _make_cfg_full():
    return Cfg(n_nodes=100000, n_cores=8, npc_raw=12500, npc=12800,
               spt=5, nb=4, ctb=3)


def kernel(**inputs):
    cfg = _make_cfg_full()
    x = np.asarray(inputs["x"], np.float32)
    ei = np.asarray(inputs["edge_index"])
    Wgs, Wl = [], []
    for L, (a, b, c, bias) in enumerate(
            [("Wg1", "Wl1", "Ws1", "b1"), ("Wg2", "Wl2", "Ws2", "b2"),
             ("Wgo", "Wlo", "Wso", "bo")]):
        bv = np.asarray(inputs[bias], np.float32)
        assert np.all(bv == 0.0), "nonzero bias not supported by this build"
        Wgs.append((np.asarray(inputs[a], np.float32) +
                    np.asarray(inputs[c], np.float32)).astype(BF_NP))
        Wl.append(np.asarray(inputs[b], np.float32))

    per_core, perm_pos = prep_host(x, ei, Wl[0], cfg)

    nc = bacc.Bacc("TRN2", target_bir_lowering=False, debug=False,
                   enable_asserts=False, num_devices=cfg.n_cores,
                   num_swdge_queues=4)
    build_program(nc, cfg)
    nc.compile()

    in_maps = []
    for c in range(cfg.n_cores):
        m = dict(per_core[c])
        for L in range(3):
            m[f"w_gs{L}"] = Wgs[L]
        for L in range(2):
            m[f"w_ln{L}"] = Wl[L + 1].astype(BF_NP)
        in_maps.append(m)

    res = run_bass_kernel_spmd(
        nc, in_maps, core_ids=list(range(cfg.n_cores)),
        trace=bool(int(os.environ.get("GNN_TRACE", "0"))),
    )
    full = np.zeros((cfg.n_nodes, D), np.float32)
    for c in range(cfg.n_cores):
        lo = c * cfg.npc_raw
        hi = min((c + 1) * cfg.npc_raw, cfg.n_nodes)
        full[lo:hi] = res.results[c]["out_own"][perm_pos[lo:hi]]
    kernel.last_results = res
    return full.astype(np.float32)
